# revision 10
# baseline (speedup 1.0000x reference)
"""Trainium2 Bass kernel for nn_MultiHeadAttention_67379446939752.

Per-token multi-head attention:
  Q = q @ Wq.T + bq ; K,V likewise        [B,S,D] -> [B,S,H,HD]
  score[t,h,g] = sum_d Q[t,h,d] K[t,g,d]  (per-token HxH gram, no seq mixing)
  attn[t] = softmax(score[t]) @ V[t]      -> [B,S,D]
  out = attn @ Wo.T + bo

Strategy: data-parallel over the 16384 tokens across 8 NeuronCores (2048
tokens/core).  All big matmuls run in float32r (full PE rate at N>=256,
~1e-4 relative error).  Host pre-transposes activations/weights so the
contraction dim lands on SBUF partitions with no on-device transposes.
The per-token 16x16 attention is computed 8 tokens at a time as a single
128x128x128 matmul whose cross-token blocks are pushed to -1024 in PSUM by
a rank-8 bf16 mask matmul; exp() then zeroes them exactly.
"""
import sys
sys.path.insert(0, "/opt/trn_rl_repo")
import numpy as np
import concourse.bass as bass
import concourse.mybir as mybir
import concourse.bacc as bacc
import concourse.tile as tile
from concourse.bass_utils import run_bass_kernel_spmd

B, S, D, H, HD = 4, 4096, 2048, 16, 128
NCORES = 8
T_FULL = B * S
F32, F32R, BF16 = mybir.dt.float32, mybir.dt.float32r, mybir.dt.bfloat16
KT = D // 128            # contraction tiles
SHIFT = 25.0             # constant softmax shift (softmax-invariant)
NEG = 1024.0             # additive mask magnitude for cross-token blocks
TA = 256                 # phase-A token chunk (moving N for f32r >= 256)
TB = 256                 # phase-B token chunk
TC = 256                 # phase-C token chunk
Exp = mybir.ActivationFunctionType.Exp


def mask_consts():
    # u8[r,(t,h)] = 1 if t==r ; v8[r,(b,t',g)] = -NEG*(1 - (t'==r))
    u = np.zeros((8, 128), np.float32)
    for r in range(8):
        u[r, r * 16:(r + 1) * 16] = 1.0
    v = np.full((8, 128), -NEG, np.float32)
    for r in range(8):
        v[r, r * 16:(r + 1) * 16] = 0.0
    v = np.tile(v, (1, 4))  # [8, 512] for 4 blocks per group
    return u, v


def build(T, debug=False):
    import ml_dtypes
    TBe = min(TB, T)
    TCe = min(TC, T)
    nc = bacc.Bacc(None, target_bir_lowering=False)
    dt_in = lambda n, s: nc.dram_tensor(n, s, F32R, kind="ExternalInput")
    qT = dt_in("qT", [D, T]); kT = dt_in("kT", [D, T]); vT = dt_in("vT", [D, T])
    WqT = dt_in("WqT", [D, D]); WkT = dt_in("WkT", [D, D])
    WvT = dt_in("WvT", [D, D]); WoT = dt_in("WoT", [D, D])
    bqT = nc.dram_tensor("bqT", [128, H], F32, kind="ExternalInput")
    bkT = nc.dram_tensor("bkT", [128, H], F32, kind="ExternalInput")
    bv_row = nc.dram_tensor("bv_row", [1, D], F32R, kind="ExternalInput")
    bo_row = nc.dram_tensor("bo_row", [1, D], F32R, kind="ExternalInput")
    ones_row = nc.dram_tensor("ones_row", [1, 128], F32R, kind="ExternalInput")
    out_d = nc.dram_tensor("out", [T, D], F32, kind="ExternalOutput")
    dbg = {}
    if debug:
        for n, shp in (("dQT", [128, T * H]), ("dKT", [128, T * H]), ("dV", [T, D]), ("dATT", [D, T])):
            dbg[n] = nc.dram_tensor(n, shp, F32, kind="ExternalOutput")

    u8_np, v8_np = mask_consts()
    u8_d = nc.inline_tensor(u8_np.astype(ml_dtypes.bfloat16), "u8c")
    v8_d = nc.inline_tensor(v8_np.astype(ml_dtypes.bfloat16), "v8c")
    id_d = nc.inline_tensor(np.eye(128, dtype=np.float32), "id128")

    with tile.TileContext(nc) as tc:
        with (
            tc.tile_pool(name="dram", bufs=1, space="DRAM") as dpool,
            tc.tile_pool(name="const", bufs=1) as cpool,
        ):
            QT_d = dpool.tile([128, T * H], F32)   # col = t*16+h, row = d
            KT_d = dpool.tile([128, T * H], F32)
            V_d = dpool.tile([T, D], F32)
            ATT_d = dpool.tile([D, T], F32R)

            u8 = cpool.tile([8, 128], BF16, tag="u8")
            v8 = cpool.tile([8, 512], BF16, tag="v8")
            ident = cpool.tile([128, 128], F32, tag="ident")
            nc.sync.dma_start(u8[:], u8_d[:])
            nc.sync.dma_start(v8[:], v8_d[:])
            nc.sync.dma_start(ident[:], id_d[:])
            biasq = cpool.tile([128, H], F32, tag="bq")
            biask = cpool.tile([128, H], F32, tag="bk")
            bvr = cpool.tile([1, D], F32R, tag="bv")
            bor = cpool.tile([1, D], F32R, tag="bo")
            onesr = cpool.tile([1, 128], F32R, tag="ones")
            nc.sync.dma_start(biasq[:], bqT[:])
            nc.sync.dma_start(biask[:], bkT[:])
            nc.sync.dma_start(bvr[:], bv_row[:])
            nc.sync.dma_start(bor[:], bo_row[:])
            nc.sync.dma_start(onesr[:], ones_row[:])
            shiftc = cpool.tile([128, 1], F32, tag="shiftc")
            nc.vector.memset(shiftc[:], -SHIFT)

            # ---------------- Phase A: projections ----------------
            for xin, win, bias, spill, natural in (
                (qT, WqT, biasq, QT_d, False),
                (kT, WkT, biask, KT_d, False),
                (vT, WvT, None, V_d, True),
            ):
                with (
                    tc.tile_pool(name="wt", bufs=1) as wpool,
                    tc.tile_pool(name="xs", bufs=2) as xpool,
                    tc.tile_pool(name="psA", bufs=8, space="PSUM") as psA,
                    tc.tile_pool(name="stA", bufs=(4 if natural else 1)) as stA,
                ):
                    wt = wpool.tile([128, KT, D], F32R, tag="wt")
                    nc.sync.dma_start(wt[:], win.ap().rearrange("(it p) j -> p it j", p=128))
                    for c in range(T // TA):
                        xs = xpool.tile([128, KT, TA], F32R, tag="xs")
                        nc.sync.dma_start(
                            xs[:], xin[:, c * TA:(c + 1) * TA].rearrange("(it p) t -> p it t", p=128))
                        if not natural:
                            # out tiles [d, t] per head; staged (t,h)-interleaved
                            stg = stA.tile([128, TA, H], F32, tag="stA")
                            for jt in range(D // 128):
                                ps = psA.tile([128, TA], F32, tag="psA")
                                for ki in range(KT):
                                    nc.tensor.matmul(
                                        ps[:], wt[:, ki, jt * 128:(jt + 1) * 128],
                                        xs[:, ki, :], start=(ki == 0), stop=(ki == KT - 1))
                                nc.any.tensor_scalar_add(stg[:, :, jt], ps[:], bias[:, jt:jt + 1])
                            nc.sync.dma_start(
                                spill[:, c * TA * H:(c + 1) * TA * H],
                                stg[:].rearrange("p t h -> p (t h)"))
                        else:
                            # natural [t, j] : lhsT = x^T tile, rhs = W^T chunk
                            for tt in range(TA // 128):
                                for jc in range(D // 512):
                                    ps = psA.tile([128, 512], F32, tag="psAv")
                                    for ki in range(KT):
                                        nc.tensor.matmul(
                                            ps[:], xs[:, ki, tt * 128:(tt + 1) * 128],
                                            wt[:, ki, jc * 512:(jc + 1) * 512],
                                            start=(ki == 0), stop=False)
                                    nc.tensor.matmul(ps[:], onesr[:], bvr[:, jc * 512:(jc + 1) * 512],
                                                     start=False, stop=True)
                                    st = stA.tile([128, 512], F32, tag="stAv")
                                    nc.any.tensor_copy(st[:], ps[:])
                                    nc.sync.dma_start(
                                        spill[c * TA + tt * 128: c * TA + (tt + 1) * 128,
                                              jc * 512:(jc + 1) * 512], st[:])

            # ---------------- Phase B: per-token attention ----------------
            with (
                tc.tile_pool(name="qk", bufs=2) as qkpool,
                tc.tile_pool(name="vb", bufs=2) as vpool,
                tc.tile_pool(name="attc", bufs=2) as apool,
                tc.tile_pool(name="eb", bufs=3) as epool,
                tc.tile_pool(name="zb", bufs=4) as zpool,
                tc.tile_pool(name="psS", bufs=4, space="PSUM") as psS,
                tc.tile_pool(name="psT", bufs=2, space="PSUM") as psT,
                tc.tile_pool(name="psA2", bufs=2, space="PSUM") as psA2,
            ):
                for c in range(T // TBe):
                    t0c = c * TBe
                    QTs = qkpool.tile([128, TBe, H], F32, tag="QTs")
                    KTs = qkpool.tile([128, TBe, H], F32, tag="KTs")
                    nc.sync.dma_start(
                        QTs[:], QT_d[:, t0c * H:(t0c + TBe) * H].rearrange(
                            "p (t h) -> p t h", h=H))
                    nc.sync.dma_start(
                        KTs[:], KT_d[:, t0c * H:(t0c + TBe) * H].rearrange(
                            "p (t h) -> p t h", h=H))
                    Vs = vpool.tile([128, TBe // 8, 128], F32, tag="Vs")
                    for tp in range(8):
                        nc.sync.dma_start(
                            Vs[tp * 16:(tp + 1) * 16, :, :],
                            V_d[t0c + tp: t0c + TBe: 8, :].rearrange(
                                "b (g d) -> g b d", g=16))
                    ATTc = apool.tile([128, H, TBe], F32R, tag="ATTc")
                    for grp in range(TBe // 32):
                        t0 = grp * 32
                        pss = []
                        for b in range(4):
                            sl = slice(t0 + b * 8, t0 + (b + 1) * 8)
                            ps_b = psS.tile([128, 128], F32, tag="ps_s")
                            nc.tensor.matmul(
                                ps_b[:],
                                QTs[:, sl, :].rearrange("p t h -> p (t h)"),
                                KTs[:, sl, :].rearrange("p t h -> p (t h)"),
                                start=True, stop=False)
                            nc.tensor.matmul(ps_b[:], u8[:], v8[:, 0:128],
                                             start=False, stop=True)
                            pss.append(ps_b)
                        E = epool.tile([128, 512], F32, tag="E")
                        Z = zpool.tile([128, 4], F32, tag="Z")
                        for b in range(4):
                            nc.scalar.activation(
                                E[:, b * 128:(b + 1) * 128], pss[b][:],
                                Exp, bias=shiftc[:], accum_out=Z[:, b:b + 1])
                        R = zpool.tile([128, 4], F32, tag="R")
                        nc.vector.reciprocal(R[:], Z[:])
                        Wb = epool.tile([128, 512], F32, tag="Wb")
                        for b in range(4):
                            nc.vector.tensor_scalar_mul(
                                Wb[:, b * 128:(b + 1) * 128], E[:, b * 128:(b + 1) * 128],
                                R[:, b:b + 1])
                        WTs = epool.tile([128, 512], F32, tag="WTs")
                        for b in range(4):
                            ps_t = psT.tile([128, 128], F32, tag="ps_t")
                            nc.tensor.transpose(
                                ps_t[:], Wb[:, b * 128:(b + 1) * 128], ident[:])
                            nc.any.tensor_copy(WTs[:, b * 128:(b + 1) * 128], ps_t[:])
                        for b in range(4):
                            ps_a = psA2.tile([128, 128], F32, tag="ps_a")
                            nc.tensor.matmul(
                                ps_a[:], Vs[:, grp * 4 + b, :],
                                WTs[:, b * 128:(b + 1) * 128],
                                start=True, stop=True)
                            nc.any.tensor_copy(
                                ATTc[:, :, t0 + b * 8: t0 + (b + 1) * 8].rearrange(
                                    "p h t -> p t h"),
                                ps_a[:].rearrange("p (t h) -> p t h", t=8))
                    nc.sync.dma_start(
                        ATT_d[:, t0c:t0c + TBe].rearrange("(h p) t -> p h t", p=128), ATTc[:])

            # ---------------- Phase C: output projection ----------------
            with (
                tc.tile_pool(name="wo", bufs=1) as wopool,
                tc.tile_pool(name="ca", bufs=2) as capool,
                tc.tile_pool(name="psC", bufs=4, space="PSUM") as psC,
                tc.tile_pool(name="stC", bufs=4) as stC,
            ):
                wo = wopool.tile([128, H, D], F32R, tag="wo")
                nc.sync.dma_start(wo[:], WoT.ap().rearrange("(h p) j -> p h j", p=128))
                for cc in range(T // TCe):
                    ATTs = capool.tile([128, H, TCe], F32R, tag="ATTs")
                    nc.sync.dma_start(
                        ATTs[:], ATT_d[:, cc * TCe:(cc + 1) * TCe].rearrange(
                            "(h p) t -> p h t", p=128))
                    for tt in range(TCe // 128):
                        for jc in range(D // 512):
                            ps = psC.tile([128, 512], F32, tag="psC")
                            for h in range(H):
                                nc.tensor.matmul(
                                    ps[:], ATTs[:, h, tt * 128:(tt + 1) * 128],
                                    wo[:, h, jc * 512:(jc + 1) * 512],
                                    start=(h == 0), stop=False)
                            nc.tensor.matmul(ps[:], onesr[:], bor[:, jc * 512:(jc + 1) * 512],
                                             start=False, stop=True)
                            st = stC.tile([128, 512], F32, tag="stC")
                            nc.any.tensor_copy(st[:], ps[:])
                            nc.sync.dma_start(
                                out_d[cc * TCe + tt * 128: cc * TCe + (tt + 1) * 128,
                                      jc * 512:(jc + 1) * 512], st[:])
            if debug:
                with tc.tile_pool(name="dbgp", bufs=2) as dbgp:
                    for src, name in ((QT_d, "dQT"), (KT_d, "dKT"), (V_d, "dV"), (ATT_d, "dATT")):
                        for r0 in range(0, src.shape[0], 128):
                            tcopy = dbgp.tile([128, src.shape[1]], F32, tag="dbg")
                            nc.sync.dma_start(tcopy[:], src[r0:r0 + 128, :].bitcast(F32))
                            nc.sync.dma_start(dbg[name][r0:r0 + 128, :], tcopy[:])
    nc.compile()
    return nc


_cache = {}


def get_nc(T):
    if T not in _cache:
        _cache[T] = build(T)
    return _cache[T]


def make_in_maps(q, k, v, Wq, bq, Wk, bk, Wv, bv, Wo, bo, ncores=NCORES, T=None):
    f = np.float32
    q = np.asarray(q, f).reshape(-1, D)
    k = np.asarray(k, f).reshape(-1, D)
    v = np.asarray(v, f).reshape(-1, D)
    if T is None:
        T = q.shape[0] // ncores
    WqT = np.ascontiguousarray(np.asarray(Wq, f).T)
    WkT = np.ascontiguousarray(np.asarray(Wk, f).T)
    WvT = np.ascontiguousarray(np.asarray(Wv, f).T)
    WoT = np.ascontiguousarray(np.asarray(Wo, f).T)
    bqT = np.ascontiguousarray(np.asarray(bq, f).reshape(H, 128).T)
    bkT = np.ascontiguousarray(np.asarray(bk, f).reshape(H, 128).T)
    bvr = np.asarray(bv, f).reshape(1, D)
    bor = np.asarray(bo, f).reshape(1, D)
    maps = []
    for c in range(ncores):
        sl = slice(c * T, (c + 1) * T)
        maps.append({
            "qT": np.ascontiguousarray(q[sl].T),
            "kT": np.ascontiguousarray(k[sl].T),
            "vT": np.ascontiguousarray(v[sl].T),
            "WqT": WqT, "WkT": WkT, "WvT": WvT, "WoT": WoT,
            "bqT": bqT, "bkT": bkT, "bv_row": bvr, "bo_row": bor,
            "ones_row": np.ones((1, 128), f),
        })
    return maps, T


def kernel(q, k, v, Wq, bq, Wk, bk, Wv, bv, Wo, bo):
    maps, T = make_in_maps(q, k, v, Wq, bq, Wk, bk, Wv, bv, Wo, bo)
    nc = get_nc(T)
    res = run_bass_kernel_spmd(nc, maps, list(range(NCORES)))
    out = np.concatenate([np.asarray(r["out"]) for r in res.results], axis=0)
    return out.reshape(B, S, D).astype(np.float32)


# revision 24
# speedup vs baseline: 2.0841x; 2.0841x over previous
"""Trainium2 Bass kernel for nn_MultiHeadAttention_67379446939752.

Per-token multi-head attention:
  Q = q @ Wq.T + bq ; K,V likewise        [B,S,D] -> [B,S,H,HD]
  score[t,h,g] = sum_d Q[t,h,d] K[t,g,d]  (per-token HxH gram, no seq mixing)
  attn[t] = softmax(score[t]) @ V[t]      -> [B,S,D]
  out = attn @ Wo.T + bo

Strategy: data-parallel over the 16384 tokens across 8 NeuronCores (2048
tokens/core).  All big matmuls run in float32r (full PE rate at N>=256,
~1e-4 relative error).  Host pre-transposes activations/weights so the
contraction dim lands on SBUF partitions with no on-device transposes.
The per-token 16x16 attention is computed 8 tokens at a time as a single
128x128x128 fp32 matmul whose cross-token blocks are pushed to -1024 in
PSUM by a rank-8 bf16 mask matmul; exp() then zeroes them exactly, so the
block-diagonal softmax needs no masking pass on DVE.
"""
import sys
sys.path.insert(0, "/opt/trn_rl_repo")
import numpy as np
import concourse.bass as bass
import concourse.mybir as mybir
import concourse.bacc as bacc
import concourse.tile as tile
from concourse.bass_utils import run_bass_kernel_spmd

B, S, D, H, HD = 4, 4096, 2048, 16, 128
NCORES = 8
T_FULL = B * S
F32, F32R, BF16 = mybir.dt.float32, mybir.dt.float32r, mybir.dt.bfloat16
KT = D // 128            # contraction tiles
SHIFT = 25.0             # constant softmax shift (softmax-invariant)
NEG = 1024.0             # additive mask magnitude for cross-token blocks
TA = 256                 # token chunk (phase A/B/C share this granularity)
Exp = mybir.ActivationFunctionType.Exp


def mask_consts():
    # u8[r,(t,h)] = 1 if t==r ; v8[r,(t',g)] = -NEG*(1 - (t'==r))
    u = np.zeros((8, 128), np.float32)
    for r in range(8):
        u[r, r * 16:(r + 1) * 16] = 1.0
    v = np.full((8, 128), -NEG, np.float32)
    for r in range(8):
        v[r, r * 16:(r + 1) * 16] = 0.0
    return u, v


def build(T, debug=False, repeat=1, trace_sim=False):
    import ml_dtypes
    TAe = min(TA, T)
    NCH = T // TAe           # chunks
    NBK = TAe // 8           # 8-token blocks per chunk
    nc = bacc.Bacc(None, target_bir_lowering=False)
    dt_in = lambda n, s: nc.dram_tensor(n, s, F32R, kind="ExternalInput")
    qT = dt_in("qT", [D, T]); kT = dt_in("kT", [D, T]); vT = dt_in("vT", [D, T])
    WqT = dt_in("WqT", [D, D]); WkT = dt_in("WkT", [D, D])
    WvT = dt_in("WvT", [D, D]); WoT = dt_in("WoT", [D, D])
    bqT = nc.dram_tensor("bqT", [128, H], F32, kind="ExternalInput")
    bkT = nc.dram_tensor("bkT", [128, H], F32, kind="ExternalInput")
    bvT = nc.dram_tensor("bvT", [128, H], F32, kind="ExternalInput")
    bo_row = nc.dram_tensor("bo_row", [1, D], F32R, kind="ExternalInput")
    ones_row = nc.dram_tensor("ones_row", [1, 128], F32R, kind="ExternalInput")
    out_d = nc.dram_tensor("out", [T, D], F32, kind="ExternalOutput")
    dbg = {}
    if debug:
        for n, shp in (("dQT", [128, T * H]), ("dKT", [128, T * H]),
                       ("dV", [128, T * H]), ("dATT", [D, T])):
            dbg[n] = nc.dram_tensor(n, shp, F32, kind="ExternalOutput")

    u8_np, v8_np = mask_consts()
    u8_d = nc.inline_tensor(u8_np.astype(ml_dtypes.bfloat16), "u8c")
    v8_d = nc.inline_tensor(v8_np.astype(ml_dtypes.bfloat16), "v8c")
    id_d = nc.inline_tensor(np.eye(128, dtype=np.float32), "id128").bitcast(F32R)

    with tile.TileContext(nc, trace_sim=trace_sim) as tc:
        with (
            tc.tile_pool(name="dram", bufs=1, space="DRAM") as dpool,
            tc.tile_pool(name="const", bufs=1) as cpool,
        ):
            # per-chunk spill tiles (fine-grained cross-phase deps)
            QT_ds = [dpool.tile([128, TAe * H], F32R, tag=f"QTd{i}", name=f"QTd{i}") for i in range(NCH)]
            KT_ds = [dpool.tile([128, TAe * H], F32R, tag=f"KTd{i}", name=f"KTd{i}") for i in range(NCH)]
            VT_ds = [dpool.tile([128, TAe * H], F32R, tag=f"VTd{i}", name=f"VTd{i}") for i in range(NCH)]
            ATT_ds = [dpool.tile([D, TAe], F32R, tag=f"ATTd{i}", name=f"ATTd{i}") for i in range(NCH)]

            u8 = cpool.tile([8, 128], BF16, tag="u8")
            v8 = cpool.tile([8, 128], BF16, tag="v8")
            ident = cpool.tile([128, 128], F32R, tag="ident")
            nc.sync.dma_start(u8[:], u8_d[:])
            nc.sync.dma_start(v8[:], v8_d[:])
            nc.sync.dma_start(ident[:], id_d[:])
            biasq = cpool.tile([128, H], F32, tag="bq")
            biask = cpool.tile([128, H], F32, tag="bk")
            biasv = cpool.tile([128, H], F32, tag="bvt")
            bor = cpool.tile([1, D], F32R, tag="bo")
            onesr = cpool.tile([1, 128], F32R, tag="ones")
            nc.sync.dma_start(biasq[:], bqT[:])
            nc.sync.dma_start(biask[:], bkT[:])
            nc.sync.dma_start(biasv[:], bvT[:])
            nc.sync.dma_start(bor[:], bo_row[:])
            nc.sync.dma_start(onesr[:], ones_row[:])
            shiftc = cpool.tile([128, 1], F32, tag="shiftc")
            nc.vector.memset(shiftc[:], -SHIFT)

            def _load_w(pool, win, tag):
                # separate quarter tiles -> first matmuls start after 1/4 load
                src = win.ap().rearrange("(it p) j -> p it j", p=128)
                parts = []
                for q in range(4):
                    wq = pool.tile([128, 4, D], F32R, tag=f"{tag}{q}", name=f"{tag}{q}")
                    nc.sync.dma_start(wq[:], src[:, q * 4:(q + 1) * 4, :])
                    parts.append(wq)
                return parts

            def _phases():
                # ---------------- Phase A: projections ----------------
                with (
                    tc.tile_pool(name="wt", bufs=1) as wpool,
                    tc.tile_pool(name="xs", bufs=2) as xpool,
                    tc.tile_pool(name="psA", bufs=8, space="PSUM") as psA,
                    tc.tile_pool(name="stA", bufs=1) as stA,
                ):
                    for xin, win, bias, spills in (
                        (qT, WqT, biasq, QT_ds),
                        (kT, WkT, biask, KT_ds),
                        (vT, WvT, biasv, VT_ds),
                    ):
                        xs0 = xpool.tile([128, KT, TAe], F32R, tag="xs", name="xs0")
                        nc.sync.dma_start(
                            xs0[:], xin[:, 0:TAe].rearrange("(it p) t -> p it t", p=128))
                        wt = _load_w(wpool, win, "wt")
                        for c in range(NCH):
                            if c == 0:
                                xs = xs0
                            else:
                                xs = xpool.tile([128, KT, TAe], F32R, tag="xs")
                                nc.sync.dma_start(
                                    xs[:], xin[:, c * TAe:(c + 1) * TAe].rearrange(
                                        "(it p) t -> p it t", p=128))
                            stg = stA.tile([128, TAe, H], F32R, tag="stA")
                            for jh in range(2):
                                pss = [psA.tile([128, TAe], F32, tag="psA",
                                                name=f"psA{jh}_{j}") for j in range(8)]
                                for q in range(4):
                                    for jl in range(8):
                                        jt = jh * 8 + jl
                                        for kl in range(4):
                                            ki = q * 4 + kl
                                            nc.tensor.matmul(
                                                pss[jl][:], wt[q][:, kl, jt * 128:(jt + 1) * 128],
                                                xs[:, ki, :], start=(ki == 0), stop=(ki == KT - 1))
                                for jl in range(8):
                                    jt = jh * 8 + jl
                                    nc.any.tensor_scalar_add(stg[:, :, jt], pss[jl][:],
                                                             bias[:, jt:jt + 1])
                            nc.sync.dma_start(
                                spills[c][:], stg[:].rearrange("p t h -> p (t h)"))

                # ---------------- Phase B: per-token attention ----------------
                with (
                    tc.tile_pool(name="qk", bufs=2) as qkpool,
                    tc.tile_pool(name="vb", bufs=2) as vpool,
                    tc.tile_pool(name="attc", bufs=2) as apool,
                    tc.tile_pool(name="eb", bufs=4) as epool,
                    tc.tile_pool(name="zb", bufs=6) as zpool,
                    tc.tile_pool(name="psS", bufs=2, space="PSUM") as psS,
                    tc.tile_pool(name="psT", bufs=2, space="PSUM") as psT,
                    tc.tile_pool(name="psV", bufs=2, space="PSUM") as psV,
                    tc.tile_pool(name="psA2", bufs=2, space="PSUM") as psA2,
                ):
                    for c in range(NCH):
                        t0c = c * TAe
                        QTs = qkpool.tile([128, TAe, H], F32R, tag="QTs")
                        KTs = qkpool.tile([128, TAe, H], F32R, tag="KTs")
                        nc.gpsimd.dma_start(
                            QTs[:], QT_ds[c][:].rearrange("p (t h) -> p t h", h=H))
                        nc.gpsimd.dma_start(
                            KTs[:], KT_ds[c][:].rearrange("p (t h) -> p t h", h=H))
                        VTs = vpool.tile([128, TAe, H], F32R, tag="VTs")
                        nc.sync.dma_start(
                            VTs[:], VT_ds[c][:].rearrange("p (t h) -> p t h", h=H))
                        ATTc = apool.tile([128, H, TAe], F32R, tag="ATTc")
                        for bk in range(NBK):
                            sl = slice(bk * 8, (bk + 1) * 8)
                            w0 = (bk // 2) * 2            # even-aligned 2-block window
                            off = (bk % 2) * 128          # valid column offset
                            slw = slice(w0 * 8, (w0 + 2) * 8)
                            ps_b = psS.tile([128, 256], F32, tag="ps_s")
                            nc.tensor.matmul(
                                ps_b[:],
                                QTs[:, sl, :].rearrange("p t h -> p (t h)"),
                                KTs[:, slw, :].rearrange("p t h -> p (t h)"),
                                start=True, stop=False, skip_group_check=True)
                            nc.tensor.matmul(ps_b[:, off:off + 128], u8[:], v8[:],
                                             start=False, stop=True, skip_group_check=True)
                            E = epool.tile([128, 128], F32, tag="E")
                            Z = zpool.tile([128, 1], F32, tag="Z")
                            nc.scalar.activation(E[:], ps_b[:, off:off + 128], Exp,
                                                 bias=shiftc[:], accum_out=Z[:])
                            R = zpool.tile([128, 1], F32, tag="R")
                            nc.vector.reciprocal(R[:], Z[:])
                            Wb = epool.tile([128, 128], F32R, tag="Wb")
                            nc.vector.tensor_scalar_mul(Wb[:], E[:], R[:])
                            ps_t = psT.tile([128, 128], F32R, tag="ps_t")
                            nc.tensor.transpose(ps_t[:], Wb[:], ident[:])
                            WTs = epool.tile([128, 128], F32R, tag="WTs")
                            nc.any.tensor_copy(WTs[:], ps_t[:])
                            ps_v = psV.tile([128, 128], F32R, tag="ps_v")
                            nc.tensor.transpose(
                                ps_v[:], VTs[:, sl, :].rearrange("p t h -> p (t h)"),
                                ident[:])
                            Vb = epool.tile([128, 128], F32R, tag="Vb")
                            nc.any.tensor_copy(Vb[:], ps_v[:])
                            ps_a = psA2.tile([128, 128], F32, tag="ps_a")
                            nc.tensor.matmul(ps_a[:], Vb[:], WTs[:],
                                             start=True, stop=True)
                            nc.any.tensor_copy(
                                ATTc[:, :, bk * 8:(bk + 1) * 8].rearrange("p h t -> p t h"),
                                ps_a[:].rearrange("p (t h) -> p t h", t=8))
                        nc.sync.dma_start(
                            ATT_ds[c][:].rearrange("(h p) t -> p h t", p=128), ATTc[:])

                # ---------------- Phase C: output projection ----------------
                with (
                    tc.tile_pool(name="wo", bufs=1) as wopool,
                    tc.tile_pool(name="ca", bufs=2) as capool,
                    tc.tile_pool(name="psC", bufs=8, space="PSUM") as psC,
                    tc.tile_pool(name="stC", bufs=4) as stC,
                ):
                    ATTs0 = capool.tile([128, H, TAe], F32R, tag="ATTs", name="ATTs0")
                    nc.sync.dma_start(
                        ATTs0[:], ATT_ds[0][:].rearrange("(h p) t -> p h t", p=128))
                    wo = _load_w(wopool, WoT, "wo")
                    for cc in range(NCH):
                        if cc == 0:
                            ATTs = ATTs0
                        else:
                            ATTs = capool.tile([128, H, TAe], F32R, tag="ATTs")
                            nc.sync.dma_start(
                                ATTs[:], ATT_ds[cc][:].rearrange("(h p) t -> p h t", p=128))
                        tjs = [(tt, jc) for tt in range(TAe // 128) for jc in range(D // 512)]
                        pss = [psC.tile([128, 512], F32, tag="psC", name=f"psC{i}")
                               for i in range(len(tjs))]
                        for hq in range(4):
                            for i, (tt, jc) in enumerate(tjs):
                                for hl in range(4):
                                    h = hq * 4 + hl
                                    nc.tensor.matmul(
                                        pss[i][:], ATTs[:, h, tt * 128:(tt + 1) * 128],
                                        wo[hq][:, hl, jc * 512:(jc + 1) * 512],
                                        start=(h == 0), stop=False)
                        for i, (tt, jc) in enumerate(tjs):
                            nc.tensor.matmul(pss[i][:], onesr[:], bor[:, jc * 512:(jc + 1) * 512],
                                             start=False, stop=True)
                            st = stC.tile([128, 512], F32, tag="stC")
                            nc.any.tensor_copy(st[:], pss[i][:])
                            nc.sync.dma_start(
                                out_d[cc * TAe + tt * 128: cc * TAe + (tt + 1) * 128,
                                      jc * 512:(jc + 1) * 512], st[:])

            for _rep in range(repeat):
                _phases()

            if debug:
                with tc.tile_pool(name="dbgp", bufs=2) as dbgp:
                    def dump(name, srcs, width):
                        for i, srct in enumerate(srcs):
                            flat = srct[:].rearrange("p a b -> p (a b)") \
                                if len(srct.shape) == 3 else srct[:]
                            rows = flat.shape[0]
                            for r0 in range(0, rows, 128):
                                tcp = dbgp.tile([128, width], F32, tag="dbg")
                                nc.sync.dma_start(tcp[:], flat[r0:r0 + 128, :].bitcast(F32))
                                nc.sync.dma_start(
                                    dbg[name][r0:r0 + 128, i * width:(i + 1) * width], tcp[:])
                    dump("dQT", QT_ds, TAe * H)
                    dump("dKT", KT_ds, TAe * H)
                    dump("dV", VT_ds, TAe * H)
                    dump("dATT", ATT_ds, TAe)
    nc.compile()
    return nc


_cache = {}


def get_nc(T):
    if T not in _cache:
        _cache[T] = build(T)
    return _cache[T]


def make_in_maps(q, k, v, Wq, bq, Wk, bk, Wv, bv, Wo, bo, ncores=NCORES, T=None):
    f = np.float32
    q = np.asarray(q, f).reshape(-1, D)
    k = np.asarray(k, f).reshape(-1, D)
    v = np.asarray(v, f).reshape(-1, D)
    if T is None:
        T = q.shape[0] // ncores
    WqT = np.ascontiguousarray(np.asarray(Wq, f).T)
    WkT = np.ascontiguousarray(np.asarray(Wk, f).T)
    WvT = np.ascontiguousarray(np.asarray(Wv, f).T)
    WoT = np.ascontiguousarray(np.asarray(Wo, f).T)
    bqT = np.ascontiguousarray(np.asarray(bq, f).reshape(H, 128).T)
    bkT = np.ascontiguousarray(np.asarray(bk, f).reshape(H, 128).T)
    bvTc = np.ascontiguousarray(np.asarray(bv, f).reshape(H, 128).T)
    bor = np.asarray(bo, f).reshape(1, D)
    maps = []
    for c in range(ncores):
        sl = slice(c * T, (c + 1) * T)
        maps.append({
            "qT": np.ascontiguousarray(q[sl].T),
            "kT": np.ascontiguousarray(k[sl].T),
            "vT": np.ascontiguousarray(v[sl].T),
            "WqT": WqT, "WkT": WkT, "WvT": WvT, "WoT": WoT,
            "bqT": bqT, "bkT": bkT, "bvT": bvTc, "bo_row": bor,
            "ones_row": np.ones((1, 128), f),
        })
    return maps, T


def kernel(q, k, v, Wq, bq, Wk, bk, Wv, bv, Wo, bo):
    maps, T = make_in_maps(q, k, v, Wq, bq, Wk, bk, Wv, bv, Wo, bo)
    nc = get_nc(T)
    res = run_bass_kernel_spmd(nc, maps, list(range(NCORES)))
    out = np.concatenate([np.asarray(r["out"]) for r in res.results], axis=0)
    return out.reshape(B, S, D).astype(np.float32)
